# revision 26
# baseline (speedup 1.0000x reference)
"""Distributed GCN (2-layer + readout) on 8 Trainium2 NeuronCores.

Nodes sharded 8-way by dst owner (12500 real + pad -> 12544 local rows
per core, 98 tiles of 128). Per GCN layer:

- each core computes its table shard s = dinv * (h @ W) on TensorE and
  writes it bf16 into 256B-padded rows ([NL, 128] bf16, cols 64:128
  unused) so dma_gather's 256B-granularity constraint is met,
- the shard is AllGathered in 4 groups of <=32 tiles; each group's
  gathered table (<=32768 rows) is int16-addressable, so group == chunk,
- per-edge messages stream via gpsimd.dma_gather in dst-sorted order
  (grouped per (dst-tile, src-chunk), padded to 128-multiples),
- aggregation runs on TensorE: for each 128-message block a one-hot
  routing matrix (generated on DVE via is_equal against a constant iota
  tile, bf16 2x mode) is the stationary operand of a matmul that
  accumulates messages into the dst tile's PSUM accumulator. Duplicate
  dst rows within a block are handled natively by PSUM accumulation, so
  no scatter-add (and none of its descriptor-generation serialization)
  is needed.
- epilogue per 32-tile super-phase: h = relu(dinv*(acc + s_local) + b),
  then (layer 1) the next layer's table group is built and AllGathered
  immediately, overlapping the remaining gather phases.

The one-hot tiles are laid out [128 msgs, 128 dst, NB] with the batch
dim innermost so every tensor_tensor operand keeps a packed 2-byte last
dim (DVE 2x perf mode); the matmul reads the strided [:, :, b] slice.
"""
import numpy as np
import ml_dtypes

from concourse import bass, bacc, tile, mybir, bass_utils

F32 = mybir.dt.float32
BF16 = mybir.dt.bfloat16
I16 = mybir.dt.int16

NCORES = 8
D = 64
SUPER_TILES = 16          # dst tiles per PSUM super-phase (2 banks, 2 bufs)
GCAP_BLOCKS = 8           # <= 1024 idxs per dma_gather (16KB/engine packet cap)
NB_OH = 16                # message blocks per one-hot batch


def _roundup(x, m=128):
    return (x + m - 1) // m * m


def preprocess(edge_index, n_nodes):
    """Build index structures + static schedule. Pure numpy."""
    src = np.asarray(edge_index[0], dtype=np.int64)
    dst = np.asarray(edge_index[1], dtype=np.int64)
    REAL = (n_nodes + NCORES - 1) // NCORES          # 12500
    NL = _roundup(REAL)                               # 12544
    G = NL // 128                                     # 98
    assert G % 2 == 0
    HALF = NL // 2                                    # 6272 (= tile G//2 start)
    HT = NCORES * HALF                                # rows per half table
    nch = -(-HT // 32768)                             # int16 chunks per half
    CH = HT // nch                                    # 25088
    assert HT % nch == 0 and CH <= 32768

    # PSUM super-phases over dst tiles
    supers = []
    t = 0
    while t < G:
        nt = min(SUPER_TILES, G - t)
        supers.append((t, nt))
        t += nt
    NSUP = len(supers)
    NCHUNK = 2 * nch

    # dst side
    owner = dst // REAL
    loc = dst % REAL
    tile_e = loc // 128
    row_e = loc % 128

    # src side: position inside the per-half AllGather output
    so = src // REAL
    sloc = src % REAL
    s_h = sloc // HALF
    s_pos = so * HALF + (sloc % HALF)
    chunk_e = s_h * nch + s_pos // CH
    pos = s_pos % CH
    assert pos.max() <= 32767

    deg = np.bincount(dst, minlength=n_nodes).astype(np.float32) + 1.0

    # per-(tile, chunk) group sizes: shared across cores (max, padded)
    counts = np.zeros((NCORES, G, NCHUNK), np.int64)
    np.add.at(counts, (owner, tile_e, chunk_e), 1)
    sizes = _roundup(counts.max(axis=0), 128)         # [G, NCHUNK]

    # stream order: for super -> for chunk -> for tile
    order = [(t0 + i, c)
             for (t0, nt) in supers
             for c in range(NCHUNK)
             for i in range(nt)]
    base = np.zeros((G, NCHUNK), np.int64)
    off = 0
    for (tt, cc) in order:
        base[tt, cc] = off
        off += sizes[tt, cc]
    TOT = int(off)
    NBLK = TOT // 128

    # gather instruction split per phase (super, chunk)
    phase_gathers = []  # [sup][c] -> list of (blk0, nblk)
    for (t0, nt) in supers:
        per_c = []
        for c in range(NCHUNK):
            b0 = base[t0, c] // 128
            nb = int(sizes[t0:t0 + nt, c].sum()) // 128
            instrs = []
            while nb > 0:
                n = min(GCAP_BLOCKS, nb)
                instrs.append((int(b0), int(n)))
                b0 += n
                nb -= n
            per_c.append(instrs)
        phase_gathers.append(per_c)

    # per-(tile, chunk): block range; per-tile first/last chunk w/ blocks
    blk0_tc = base // 128
    nblk_tc = sizes // 128
    # every (tile, chunk) cell must be non-empty so the per-phase PSUM
    # drain (which adds all tiles of the phase) never reads stale slots
    assert nblk_tc.min() >= 1, "empty (tile, chunk) group"

    # per-core index/dstrow streams
    gidx_all, drow_all = [], []
    for c in range(NCORES):
        m = owner == c
        tl, ch, rw, ix = tile_e[m], chunk_e[m], row_e[m], pos[m]
        key = tl * NCHUNK + ch
        o = np.argsort(key, kind="stable")
        ks = key[o]
        rank = np.arange(len(ks)) - np.searchsorted(ks, ks)
        slot = base[tl[o], ch[o]] + rank
        gflat = np.zeros(TOT, np.int16)
        dflat = np.full(TOT, 255.0, np.float32)
        gflat[slot] = ix[o].astype(np.int16)
        dflat[slot] = rw[o].astype(np.float32)
        gw = np.tile(gflat.reshape(-1, 16).T, (8, 1))          # [128, TOT//16]
        dw = np.ascontiguousarray(
            dflat.reshape(-1, 128).T).astype(ml_dtypes.bfloat16)  # [128, NBLK]
        gidx_all.append(np.ascontiguousarray(gw))
        drow_all.append(dw)

    deg_tiles = []
    for c in range(NCORES):
        d = np.ones(NL, np.float32)
        lo, hi = c * REAL, min((c + 1) * REAL, n_nodes)
        d[:hi - lo] = deg[lo:hi]
        deg_tiles.append(np.ascontiguousarray(d.reshape(G, 128).T))  # [128, G]

    meta = dict(REAL=REAL, NL=NL, G=G, HALF=HALF, CH=CH, nch=nch,
                NCHUNK=NCHUNK, TOT=TOT, NBLK=NBLK, supers=supers,
                phase_gathers=phase_gathers,
                blk0_tc=blk0_tc, nblk_tc=nblk_tc)
    return meta, gidx_all, drow_all, deg_tiles


def build(meta):
    NL, G, HALF, CH, nch, NCHUNK, TOT, NBLK = (
        meta["NL"], meta["G"], meta["HALF"], meta["CH"], meta["nch"],
        meta["NCHUNK"], meta["TOT"], meta["NBLK"])
    supers = meta["supers"]
    phase_gathers = meta["phase_gathers"]
    blk0_tc, nblk_tc = meta["blk0_tc"], meta["nblk_tc"]
    MAXNT = max(nt for (_, nt) in supers)
    GH = G // 2

    nc = bacc.Bacc("TRN2", target_bir_lowering=False, debug=False,
                   num_devices=NCORES, num_swdge_queues=4,
                   dynamic_dma_scratch_size=32768)

    xT = nc.dram_tensor("xT", [D, NL], F32, kind="ExternalInput")
    W1 = nc.dram_tensor("W1", [D, D], F32, kind="ExternalInput")
    W2 = nc.dram_tensor("W2", [D, D], F32, kind="ExternalInput")
    b1e = nc.dram_tensor("b1bc", [128, D], F32, kind="ExternalInput")
    b2e = nc.dram_tensor("b2bc", [128, D], F32, kind="ExternalInput")
    woute = nc.dram_tensor("woutbc", [128, D], F32, kind="ExternalInput")
    boute = nc.dram_tensor("boutbc", [128, 1], F32, kind="ExternalInput")
    dege = nc.dram_tensor("deg", [128, G], F32, kind="ExternalInput")
    gidxe = nc.dram_tensor("gidx", [128, TOT // 16], I16, kind="ExternalInput")
    drowe = nc.dram_tensor("drow", [128, NBLK], BF16, kind="ExternalInput")
    iotae = nc.dram_tensor("iota", [128, 128 * NB_OH], BF16,
                           kind="ExternalInput")
    idente = nc.dram_tensor("ident", [128, 128], F32, kind="ExternalInput")
    oute = nc.dram_tensor("out", [128, G], F32, kind="ExternalOutput")

    ag_in = [[nc.dram_tensor(f"ag_in{L}_{h}", [HALF, 128], BF16)
              for h in (0, 1)] for L in (0, 1)]
    ag_out = [[nc.dram_tensor(f"ag_out{L}_{h}", [NCORES * HALF, 128],
                              BF16, addr_space="Shared")
               for h in (0, 1)] for L in (0, 1)]

    def ag_in_view(L, h):
        return ag_in[L][h].ap().rearrange("(t p) d -> p t d", p=128)

    def chunk_view(L, c):
        h, j = c // nch, c % nch
        return ag_out[L][h].ap()[j * CH:(j + 1) * CH, :]

    with tile.TileContext(nc) as tc:
        with (
            tc.tile_pool(name="pool", bufs=1) as pool,
            tc.tile_pool(name="xs", bufs=2) as xsp,
            tc.tile_pool(name="msg", bufs=24) as msgp,
            tc.tile_pool(name="oh", bufs=6) as ohp,
            tc.tile_pool(name="a1", bufs=3) as a1p,
            tc.tile_pool(name="pacc", bufs=2, space="PSUM") as psum_acc,
            tc.tile_pool(name="ps", bufs=2, space="PSUM") as psum_s,
            tc.tile_pool(name="ptp", bufs=2, space="PSUM") as psum_tp,
        ):
            gidx_t = pool.tile([128, TOT // 16], I16, tag="gidx")
            drow_t = pool.tile([128, NBLK], BF16, tag="drow")
            iota_t = pool.tile([128, 128 * NB_OH], BF16, tag="iota")
            nc.scalar.dma_start(out=gidx_t[:], in_=gidxe[:])
            nc.scalar.dma_start(out=drow_t[:], in_=drowe[:])
            nc.scalar.dma_start(out=iota_t[:], in_=iotae[:])
            W1_t = pool.tile([D, D], F32, tag="w1")
            W2_t = pool.tile([D, D], F32, tag="w2")
            nc.scalar.dma_start(out=W1_t[:], in_=W1[:])
            nc.scalar.dma_start(out=W2_t[:], in_=W2[:])
            b1_t = pool.tile([128, D], F32, tag="b1")
            b2_t = pool.tile([128, D], F32, tag="b2")
            wout_t = pool.tile([128, D], F32, tag="wout")
            bout_t = pool.tile([128, 1], F32, tag="bout")
            ident_t = pool.tile([128, 128], F32, tag="ident")
            nc.scalar.dma_start(out=b1_t[:], in_=b1e[:])
            nc.scalar.dma_start(out=b2_t[:], in_=b2e[:])
            nc.scalar.dma_start(out=wout_t[:], in_=woute[:])
            nc.scalar.dma_start(out=bout_t[:], in_=boute[:])
            nc.scalar.dma_start(out=ident_t[:], in_=idente[:])
            deg_t = pool.tile([128, G], F32, tag="deg")
            nc.sync.dma_start(out=deg_t[:], in_=dege[:])
            dinv_t = pool.tile([128, G], F32, tag="dinv")
            nc.scalar.activation(dinv_t[:], deg_t[:],
                                 mybir.ActivationFunctionType.Sqrt)
            nc.vector.reciprocal(dinv_t[:], dinv_t[:])

            s_t = [pool.tile([128, G, 128], BF16, tag=f"s{L}", name=f"s{L}")
                   for L in (0, 1)]
            nc.vector.memset(s_t[0][:], 0.0)
            nc.vector.memset(s_t[1][:], 0.0)
            h_t = pool.tile([128, G, D], F32, tag="h")
            o_t = pool.tile([128, G], F32, tag="o")

            iota_v = iota_t[:].rearrange("p (m b) -> p m b", b=NB_OH)

            def build_table_l0(h):
                t0, nt = h * GH, GH
                for bt in range(t0, t0 + nt, 8):
                    bn = min(8, t0 + nt - bt)
                    xt = xsp.tile([D, 8 * 128], F32, tag="xT")
                    nc.sync.dma_start(out=xt[:, :bn * 128],
                                      in_=xT[:, bt * 128:(bt + bn) * 128])
                    pt = psum_s.tile([128, 512], F32, tag="s")
                    for k in range(bn):
                        nc.tensor.matmul(pt[:, k * D:(k + 1) * D],
                                         xt[:, k * 128:(k + 1) * 128],
                                         W1_t[:])
                    for k in range(bn):
                        nc.vector.tensor_scalar_mul(
                            s_t[0][:, bt + k, 0:D],
                            pt[:, k * D:(k + 1) * D],
                            dinv_t[:, bt + k:bt + k + 1])
                nc.sync.dma_start(out=ag_in_view(0, h),
                                  in_=s_t[0][:, t0:t0 + nt, :])
                nc.gpsimd.collective_compute(
                    "AllGather", mybir.AluOpType.bypass,
                    replica_groups=[list(range(NCORES))],
                    ins=[ag_in[0][h].ap().opt()],
                    outs=[ag_out[0][h].ap().opt()])

            def build_table_l1(h):
                t0, nt = h * GH, GH
                for k in range(nt):
                    t = t0 + k
                    tp = psum_tp.tile([64, 128], F32, tag="tp")
                    nc.tensor.transpose(tp[:], h_t[:, t, :], ident_t[:])
                    a1T = a1p.tile([64, 128], F32, tag="a1T")
                    nc.vector.tensor_copy(a1T[:], tp[:])
                    pt2 = psum_s.tile([128, 512], F32, tag="s")
                    nc.tensor.matmul(pt2[:, 0:D], a1T[:], W2_t[:])
                    nc.vector.tensor_scalar_mul(
                        s_t[1][:, t, 0:D], pt2[:, 0:D],
                        dinv_t[:, t:t + 1])
                nc.sync.dma_start(out=ag_in_view(1, h),
                                  in_=s_t[1][:, t0:t0 + nt, :])
                nc.gpsimd.collective_compute(
                    "AllGather", mybir.AluOpType.bypass,
                    replica_groups=[list(range(NCORES))],
                    ins=[ag_in[1][h].ap().opt()],
                    outs=[ag_out[1][h].ap().opt()])

            gq = [0]
            oh_tiles = {}

            def get_oh(bi):
                """One-hot batch for global block bi; generate on first use."""
                k = bi // NB_OH
                if k not in oh_tiles:
                    oh = ohp.tile([128, 128, NB_OH], BF16, tag="oh")
                    nb = min(NB_OH, NBLK - k * NB_OH)
                    dr = drow_t[:, k * NB_OH:k * NB_OH + nb]
                    nc.vector.tensor_tensor(
                        oh[:, :, :nb],
                        iota_v[:, :, :nb],
                        dr.unsqueeze(1).broadcast_to([128, 128, nb]),
                        mybir.AluOpType.is_equal)
                    oh_tiles[k] = oh
                return oh_tiles[k][:, :, bi % NB_OH]

            def agg_layer(L):
                for sup, (t0, nt) in enumerate(supers):
                    hv = h_t[:, t0:t0 + nt, :]
                    for c in range(NCHUNK):
                        # PSUM zero regions are bank-sized: accumulation
                        # groups must not interleave, so each (tile, chunk)
                        # group is contiguous and each phase drains to SBUF.
                        acc = psum_acc.tile([128, MAXNT, D], F32, tag="acc")
                        # gather this phase's messages (block-aligned)
                        mt_of = {}  # block -> (tile_ap, col)
                        for (blk0, nblk) in phase_gathers[sup][c]:
                            mt = msgp.tile([128, GCAP_BLOCKS, 128], BF16,
                                           tag="m")
                            nc.gpsimd.dma_gather(
                                mt[:, :nblk, :],
                                chunk_view(L, c),
                                gidx_t[:, blk0 * 8:(blk0 + nblk) * 8],
                                num_idxs=nblk * 128,
                                num_idxs_reg=nblk * 128,
                                elem_size=128, single_packet=True,
                                queue_num=gq[0] % 4)
                            gq[0] += 1
                            for j in range(nblk):
                                mt_of[blk0 + j] = (mt, j)
                        # hoist one-hot generation out of the matmul chain
                        for b in sorted(mt_of):
                            get_oh(b)
                        for i in range(nt):
                            t = t0 + i
                            nb = int(nblk_tc[t, c])
                            b0 = int(blk0_tc[t, c])
                            for j in range(nb):
                                mt, col = mt_of[b0 + j]
                                nc.tensor.matmul(
                                    acc[:, i, :],
                                    get_oh(b0 + j),
                                    mt[:, col, 0:D],
                                    start=(j == 0),
                                    stop=(j == nb - 1))
                        # drain phase accumulator into SBUF
                        if c == 0:
                            nc.vector.tensor_copy(hv, acc[:, :nt, :])
                        else:
                            nc.vector.tensor_tensor(hv, hv, acc[:, :nt, :],
                                                    mybir.AluOpType.add)
                    # epilogue: h = relu(dinv*(h + s_local) + b)
                    nc.vector.tensor_tensor(hv, hv,
                                            s_t[L][:, t0:t0 + nt, 0:D],
                                            mybir.AluOpType.add)
                    dvb = dinv_t[:, t0:t0 + nt].unsqueeze(2).broadcast_to(
                        [128, nt, D])
                    nc.vector.tensor_tensor(hv, hv, dvb,
                                            mybir.AluOpType.mult)
                    bias = (b1_t if L == 0 else b2_t)[:].unsqueeze(
                        1).broadcast_to([128, nt, D])
                    nc.vector.tensor_tensor(hv, hv, bias,
                                            mybir.AluOpType.add)
                    nc.scalar.activation(hv, hv,
                                         mybir.ActivationFunctionType.Relu)
                    if L == 0:
                        # launch layer-2 table half as soon as its tiles done
                        for h in (0, 1):
                            if t0 < (h + 1) * GH <= t0 + nt:
                                build_table_l1(h)
                    else:
                        wb = wout_t[:].unsqueeze(1).broadcast_to(
                            [128, nt, D])
                        nc.vector.tensor_tensor(hv, hv, wb,
                                                mybir.AluOpType.mult)
                        nc.vector.tensor_reduce(o_t[:, t0:t0 + nt], hv,
                                                axis=mybir.AxisListType.X,
                                                op=mybir.AluOpType.add)

            for h in (0, 1):
                build_table_l0(h)
            agg_layer(0)
            oh_tiles.clear()
            agg_layer(1)

            nc.vector.tensor_scalar_add(o_t[:], o_t[:], bout_t[:])
            nc.sync.dma_start(out=oute[:], in_=o_t[:])

    nc.compile()
    return nc


_CACHE = {}


def kernel(x, edge_index, batch, W1, b1, W2, b2, Wout, bout, _trace=False):
    x = np.asarray(x, np.float32)
    edge_index = np.asarray(edge_index)
    W1 = np.asarray(W1, np.float32)
    W2 = np.asarray(W2, np.float32)
    b1 = np.asarray(b1, np.float32)
    b2 = np.asarray(b2, np.float32)
    Wout = np.asarray(Wout, np.float32)
    bout = np.asarray(bout, np.float32).reshape(-1)
    N = x.shape[0]

    key = (N, edge_index.shape[1])
    if key not in _CACHE:
        meta, gidx_all, drow_all, deg_tiles = preprocess(edge_index, N)
        nc = build(meta)
        _CACHE[key] = (meta, gidx_all, drow_all, deg_tiles, nc)
    meta, gidx_all, drow_all, deg_tiles, nc = _CACHE[key]
    REAL, NL = meta["REAL"], meta["NL"]

    ident = np.eye(128, dtype=np.float32)
    b1bc = np.tile(b1[None, :], (128, 1)).astype(np.float32)
    b2bc = np.tile(b2[None, :], (128, 1)).astype(np.float32)
    woutbc = np.tile(Wout.reshape(1, -1), (128, 1)).astype(np.float32)
    boutbc = np.full((128, 1), float(bout[0]), np.float32)
    iota = np.tile(np.repeat(np.arange(128, dtype=np.float32), NB_OH)[None, :],
                   (128, 1)).astype(ml_dtypes.bfloat16)

    in_maps = []
    for c in range(NCORES):
        xs = np.zeros((NL, D), np.float32)
        lo, hi = c * REAL, min((c + 1) * REAL, N)
        xs[:hi - lo] = x[lo:hi]
        in_maps.append({
            "xT": np.ascontiguousarray(xs.T),
            "W1": W1, "W2": W2, "b1bc": b1bc, "b2bc": b2bc,
            "woutbc": woutbc, "boutbc": boutbc,
            "deg": deg_tiles[c], "gidx": gidx_all[c], "drow": drow_all[c],
            "iota": iota, "ident": ident,
        })

    res = bass_utils.run_bass_kernel_spmd(
        nc, in_maps, core_ids=list(range(NCORES)), trace=_trace)

    out = np.zeros(N, np.float32)
    for c in range(NCORES):
        o = res.results[c]["out"]
        arr = o.T.ravel()
        lo, hi = c * REAL, min((c + 1) * REAL, N)
        out[lo:hi] = arr[:hi - lo]
    if _trace:
        return out, res.exec_time_ns
    return out


# revision 37
# speedup vs baseline: 1.1007x; 1.1007x over previous
"""Distributed GCN (2-layer + readout) on 8 Trainium2 NeuronCores.

Nodes sharded 8-way by dst owner (12500 real + pad -> 12544 local rows
per core, 98 tiles of 128). Per GCN layer:

- each core computes its table shard s = dinv * (h @ W) on TensorE and
  writes it bf16 into 256B-padded rows ([NL, 128] bf16, cols 64:128
  unused) so dma_gather's 256B-granularity constraint is met,
- the shard is AllGathered in 4 groups of <=32 tiles; each group's
  gathered table (<=32768 rows) is int16-addressable, so group == chunk,
- per-edge messages stream via gpsimd.dma_gather in dst-sorted order
  (grouped per (dst-tile, src-chunk), padded to 128-multiples),
- aggregation runs on TensorE: for each 128-message block a one-hot
  routing matrix (generated on DVE via is_equal against a constant iota
  tile, bf16 2x mode) is the stationary operand of a matmul that
  accumulates messages into the dst tile's PSUM accumulator. Duplicate
  dst rows within a block are handled natively by PSUM accumulation, so
  no scatter-add (and none of its descriptor-generation serialization)
  is needed.
- epilogue per 32-tile super-phase: h = relu(dinv*(acc + s_local) + b),
  then (layer 1) the next layer's table group is built and AllGathered
  immediately, overlapping the remaining gather phases.

The one-hot tiles are laid out [128 msgs, 128 dst, NB] with the batch
dim innermost so every tensor_tensor operand keeps a packed 2-byte last
dim (DVE 2x perf mode); the matmul reads the strided [:, :, b] slice.
"""
import numpy as np
import ml_dtypes

from concourse import bass, bacc, tile, mybir, bass_utils

F32 = mybir.dt.float32
BF16 = mybir.dt.bfloat16
I16 = mybir.dt.int16

NCORES = 8
D = 64
SUPER_TILES = 16          # dst tiles per PSUM super-phase (2 banks, 2 bufs)
GCAP_BLOCKS = 8           # <= 1024 idxs per dma_gather (16KB/engine packet cap)
NB_OH = 16                # message blocks per one-hot batch


def _roundup(x, m=128):
    return (x + m - 1) // m * m


def preprocess(edge_index, n_nodes):
    """Build index structures + static schedule. Pure numpy."""
    src = np.asarray(edge_index[0], dtype=np.int64)
    dst = np.asarray(edge_index[1], dtype=np.int64)
    REAL = (n_nodes + NCORES - 1) // NCORES          # 12500
    NL = _roundup(REAL)                               # 12544
    G = NL // 128                                     # 98
    # uneven halves: half 0 ends at a super boundary so the layer-2 table's
    # first AllGather can launch as early as possible
    GH0 = (G // 2) // SUPER_TILES * SUPER_TILES
    if GH0 == 0:
        GH0 = G // 2
    GHS = [GH0, G - GH0]                              # tiles per half (48, 50)
    HALFS = [gh * 128 for gh in GHS]                  # rows per half
    # int16 chunks per half: [(half, row0, rows)]
    chunks = []
    for h, hrows in enumerate(HALFS):
        HT = NCORES * hrows
        nch_h = -(-HT // 32768)
        CHh = -(-HT // nch_h // 16) * 16
        r0 = 0
        while r0 < HT:
            chunks.append((h, r0, min(CHh, HT - r0)))
            r0 += CHh
    NCHUNK = len(chunks)
    h0_chunks = [c for c, (h, _, _) in enumerate(chunks) if h == 0]
    h1_chunks = [c for c, (h, _, _) in enumerate(chunks) if h == 1]

    # PSUM super-phases over dst tiles
    supers = []
    t = 0
    while t < G:
        nt = min(SUPER_TILES, G - t)
        supers.append((t, nt))
        t += nt

    # dst side
    owner = dst // REAL
    loc = dst % REAL
    tile_e = loc // 128
    row_e = loc % 128

    # src side: position inside the per-half AllGather output
    so = src // REAL
    sloc = src % REAL
    s_h = (sloc >= HALFS[0]).astype(np.int64)
    hrows = np.where(s_h == 0, HALFS[0], HALFS[1])
    s_pos = so * hrows + (sloc - s_h * HALFS[0])
    ch_r0 = np.array([r0 for (_, r0, _) in chunks])
    ch_h = np.array([h for (h, _, _) in chunks])
    # chunk index: first chunk of the right half whose range contains s_pos
    chunk_e = np.zeros(len(src), np.int64)
    pos = np.zeros(len(src), np.int64)
    for c, (h, r0, rows) in enumerate(chunks):
        m = (s_h == h) & (s_pos >= r0) & (s_pos < r0 + rows)
        chunk_e[m] = c
        pos[m] = s_pos[m] - r0
    assert pos.max() <= 32767

    deg = np.bincount(dst, minlength=n_nodes).astype(np.float32) + 1.0

    # per-(tile, chunk) group sizes: shared across cores (max, padded)
    counts = np.zeros((NCORES, G, NCHUNK), np.int64)
    np.add.at(counts, (owner, tile_e, chunk_e), 1)
    sizes = _roundup(counts.max(axis=0), 128)         # [G, NCHUNK]

    # stream order: two passes (all half-0 chunk phases, then half-1) so
    # each pass's gathers are gated by a single AllGather
    phase_list = [(s, c) for cs in (h0_chunks, h1_chunks)
                  for s in range(len(supers)) for c in cs]
    order = [(supers[s][0] + i, c)
             for (s, c) in phase_list
             for i in range(supers[s][1])]
    base = np.zeros((G, NCHUNK), np.int64)
    off = 0
    for (tt, cc) in order:
        base[tt, cc] = off
        off += sizes[tt, cc]
    TOT = int(off)
    NBLK = TOT // 128

    # gather instruction split per phase (super, chunk)
    phase_gathers = {}  # (sup, c) -> list of (blk0, nblk)
    for (s, c) in phase_list:
        t0, nt = supers[s]
        b0 = base[t0, c] // 128
        nb = int(sizes[t0:t0 + nt, c].sum()) // 128
        instrs = []
        while nb > 0:
            n = min(GCAP_BLOCKS, nb)
            instrs.append((int(b0), int(n)))
            b0 += n
            nb -= n
        phase_gathers[(s, c)] = instrs

    # per-(tile, chunk): block range; per-tile first/last chunk w/ blocks
    blk0_tc = base // 128
    nblk_tc = sizes // 128
    # every (tile, chunk) cell must be non-empty so the per-phase PSUM
    # drain (which adds all tiles of the phase) never reads stale slots
    assert nblk_tc.min() >= 1, "empty (tile, chunk) group"

    # per-core index/dstrow streams
    gidx_all, drow_all = [], []
    for c in range(NCORES):
        m = owner == c
        tl, ch, rw, ix = tile_e[m], chunk_e[m], row_e[m], pos[m]
        key = tl * NCHUNK + ch
        o = np.argsort(key, kind="stable")
        ks = key[o]
        rank = np.arange(len(ks)) - np.searchsorted(ks, ks)
        slot = base[tl[o], ch[o]] + rank
        gflat = np.zeros(TOT, np.int16)
        dflat = np.full(TOT, 255.0, np.float32)
        gflat[slot] = ix[o].astype(np.int16)
        dflat[slot] = rw[o].astype(np.float32)
        gw = np.tile(gflat.reshape(-1, 16).T, (8, 1))          # [128, TOT//16]
        dw = np.ascontiguousarray(
            dflat.reshape(-1, 128).T).astype(ml_dtypes.bfloat16)  # [128, NBLK]
        gidx_all.append(np.ascontiguousarray(gw))
        drow_all.append(dw)

    deg_tiles = []
    for c in range(NCORES):
        d = np.ones(NL, np.float32)
        lo, hi = c * REAL, min((c + 1) * REAL, n_nodes)
        d[:hi - lo] = deg[lo:hi]
        deg_tiles.append(np.ascontiguousarray(d.reshape(G, 128).T))  # [128, G]

    meta = dict(REAL=REAL, NL=NL, G=G, GHS=GHS, HALFS=HALFS, chunks=chunks,
                h0_chunks=h0_chunks, h1_chunks=h1_chunks,
                NCHUNK=NCHUNK, TOT=TOT, NBLK=NBLK, supers=supers,
                phase_list=phase_list, phase_gathers=phase_gathers,
                blk0_tc=blk0_tc, nblk_tc=nblk_tc)
    return meta, gidx_all, drow_all, deg_tiles


def build(meta):
    NL, G, TOT, NBLK = meta["NL"], meta["G"], meta["TOT"], meta["NBLK"]
    GHS, HALFS, chunks = meta["GHS"], meta["HALFS"], meta["chunks"]
    supers = meta["supers"]
    phase_list = meta["phase_list"]
    phase_gathers = meta["phase_gathers"]
    blk0_tc, nblk_tc = meta["blk0_tc"], meta["nblk_tc"]
    MAXNT = max(nt for (_, nt) in supers)
    half_t0 = [0, GHS[0]]

    nc = bacc.Bacc("TRN2", target_bir_lowering=False, debug=False,
                   num_devices=NCORES, num_swdge_queues=4,
                   dynamic_dma_scratch_size=32768)

    xT = nc.dram_tensor("xT", [D, NL], F32, kind="ExternalInput")
    W1 = nc.dram_tensor("W1", [D, D], F32, kind="ExternalInput")
    W2 = nc.dram_tensor("W2", [D, D], F32, kind="ExternalInput")
    b1e = nc.dram_tensor("b1bc", [128, D], F32, kind="ExternalInput")
    b2e = nc.dram_tensor("b2bc", [128, D], F32, kind="ExternalInput")
    woute = nc.dram_tensor("woutbc", [128, D], F32, kind="ExternalInput")
    boute = nc.dram_tensor("boutbc", [128, 1], F32, kind="ExternalInput")
    dege = nc.dram_tensor("deg", [128, G], F32, kind="ExternalInput")
    gidxe = nc.dram_tensor("gidx", [128, TOT // 16], I16, kind="ExternalInput")
    drowe = nc.dram_tensor("drow", [128, NBLK], BF16, kind="ExternalInput")
    iotae = nc.dram_tensor("iota", [128, 128 * NB_OH], BF16,
                           kind="ExternalInput")
    idente = nc.dram_tensor("ident", [128, 128], F32, kind="ExternalInput")
    oute = nc.dram_tensor("out", [128, G], F32, kind="ExternalOutput")

    ag_in = [[nc.dram_tensor(f"ag_in{L}_{h}", [HALFS[h], 128], BF16)
              for h in (0, 1)] for L in (0, 1)]
    ag_out = [[nc.dram_tensor(f"ag_out{L}_{h}", [NCORES * HALFS[h], 128],
                              BF16, addr_space="Shared")
               for h in (0, 1)] for L in (0, 1)]

    def ag_in_view(L, h):
        return ag_in[L][h].ap().rearrange("(t p) d -> p t d", p=128)

    def chunk_view(L, c):
        h, r0, rows = chunks[c]
        return ag_out[L][h].ap()[r0:r0 + rows, :]

    with tile.TileContext(nc) as tc:
        with (
            tc.tile_pool(name="pool", bufs=1) as pool,
            tc.tile_pool(name="xs", bufs=2) as xsp,
            tc.tile_pool(name="msg", bufs=24) as msgp,
            tc.tile_pool(name="oh", bufs=6) as ohp,
            tc.tile_pool(name="a1", bufs=3) as a1p,
            tc.tile_pool(name="pacc", bufs=2, space="PSUM") as psum_acc,
            tc.tile_pool(name="ps", bufs=2, space="PSUM") as psum_s,
            tc.tile_pool(name="ptp", bufs=2, space="PSUM") as psum_tp,
        ):
            gidx_t = pool.tile([128, TOT // 16], I16, tag="gidx")
            drow_t = pool.tile([128, NBLK], BF16, tag="drow")
            iota_t = pool.tile([128, 128 * NB_OH], BF16, tag="iota")
            nc.scalar.dma_start(out=gidx_t[:], in_=gidxe[:])
            nc.scalar.dma_start(out=drow_t[:], in_=drowe[:])
            nc.scalar.dma_start(out=iota_t[:], in_=iotae[:])
            W1_t = pool.tile([D, D], F32, tag="w1")
            W2_t = pool.tile([D, D], F32, tag="w2")
            nc.scalar.dma_start(out=W1_t[:], in_=W1[:])
            nc.scalar.dma_start(out=W2_t[:], in_=W2[:])
            b1_t = pool.tile([128, D], F32, tag="b1")
            b2_t = pool.tile([128, D], F32, tag="b2")
            wout_t = pool.tile([128, D], F32, tag="wout")
            bout_t = pool.tile([128, 1], F32, tag="bout")
            ident_t = pool.tile([128, 128], F32, tag="ident")
            nc.scalar.dma_start(out=b1_t[:], in_=b1e[:])
            nc.scalar.dma_start(out=b2_t[:], in_=b2e[:])
            nc.scalar.dma_start(out=wout_t[:], in_=woute[:])
            nc.scalar.dma_start(out=bout_t[:], in_=boute[:])
            nc.scalar.dma_start(out=ident_t[:], in_=idente[:])
            deg_t = pool.tile([128, G], F32, tag="deg")
            nc.sync.dma_start(out=deg_t[:], in_=dege[:])
            dinv_t = pool.tile([128, G], F32, tag="dinv")
            nc.scalar.activation(dinv_t[:], deg_t[:],
                                 mybir.ActivationFunctionType.Sqrt)
            nc.vector.reciprocal(dinv_t[:], dinv_t[:])

            s_t = [pool.tile([128, G, 128], BF16, tag=f"s{L}", name=f"s{L}")
                   for L in (0, 1)]
            nc.vector.memset(s_t[0][:], 0.0)
            nc.vector.memset(s_t[1][:], 0.0)
            h_t = pool.tile([128, G, D], F32, tag="h")
            o_t = pool.tile([128, G], F32, tag="o")

            iota_v = iota_t[:].rearrange("p (m b) -> p m b", b=NB_OH)

            def build_table_l0(h):
                t0, nt = half_t0[h], GHS[h]
                for bt in range(t0, t0 + nt, 8):
                    bn = min(8, t0 + nt - bt)
                    xt = xsp.tile([D, 8 * 128], F32, tag="xT")
                    nc.sync.dma_start(out=xt[:, :bn * 128],
                                      in_=xT[:, bt * 128:(bt + bn) * 128])
                    pt = psum_s.tile([128, 512], F32, tag="s")
                    for k in range(bn):
                        nc.tensor.matmul(pt[:, k * D:(k + 1) * D],
                                         xt[:, k * 128:(k + 1) * 128],
                                         W1_t[:])
                    for k in range(bn):
                        nc.vector.tensor_scalar_mul(
                            s_t[0][:, bt + k, 0:D],
                            pt[:, k * D:(k + 1) * D],
                            dinv_t[:, bt + k:bt + k + 1])
                nc.sync.dma_start(out=ag_in_view(0, h),
                                  in_=s_t[0][:, t0:t0 + nt, :])
                nc.gpsimd.collective_compute(
                    "AllGather", mybir.AluOpType.bypass,
                    replica_groups=[list(range(NCORES))],
                    ins=[ag_in[0][h].ap().opt()],
                    outs=[ag_out[0][h].ap().opt()])

            def build_table_l1(h):
                t0, nt = half_t0[h], GHS[h]
                for k in range(nt):
                    t = t0 + k
                    tp = psum_tp.tile([64, 128], F32, tag="tp")
                    nc.tensor.transpose(tp[:], h_t[:, t, :], ident_t[:])
                    a1T = a1p.tile([64, 128], F32, tag="a1T")
                    nc.vector.tensor_copy(a1T[:], tp[:])
                    pt2 = psum_s.tile([128, 512], F32, tag="s")
                    nc.tensor.matmul(pt2[:, 0:D], a1T[:], W2_t[:])
                    nc.vector.tensor_scalar_mul(
                        s_t[1][:, t, 0:D], pt2[:, 0:D],
                        dinv_t[:, t:t + 1])
                nc.sync.dma_start(out=ag_in_view(1, h),
                                  in_=s_t[1][:, t0:t0 + nt, :])
                nc.gpsimd.collective_compute(
                    "AllGather", mybir.AluOpType.bypass,
                    replica_groups=[list(range(NCORES))],
                    ins=[ag_in[1][h].ap().opt()],
                    outs=[ag_out[1][h].ap().opt()])

            gq = [0]
            oh_tiles = {}

            def get_oh(bi):
                """One-hot batch for global block bi; generate on first use."""
                k = bi // NB_OH
                if k not in oh_tiles:
                    oh = ohp.tile([128, 128, NB_OH], BF16, tag="oh")
                    nb = min(NB_OH, NBLK - k * NB_OH)
                    dr = drow_t[:, k * NB_OH:k * NB_OH + nb]
                    nc.vector.tensor_tensor(
                        oh[:, :, :nb],
                        iota_v[:, :, :nb],
                        dr.unsqueeze(1).broadcast_to([128, 128, nb]),
                        mybir.AluOpType.is_equal)
                    oh_tiles[k] = oh
                return oh_tiles[k][:, :, bi % NB_OH]

            def agg_layer(L):
                first_c, last_c = phase_list[0][1], phase_list[-1][1]
                for (sup, c) in phase_list:
                    t0, nt = supers[sup]
                    hv = h_t[:, t0:t0 + nt, :]
                    # PSUM zero regions are bank-sized: accumulation
                    # groups must not interleave, so each (tile, chunk)
                    # group is contiguous and each phase drains to SBUF.
                    acc = psum_acc.tile([128, MAXNT, D], F32, tag="acc")
                    # gather this phase's messages (block-aligned)
                    mt_of = {}  # block -> (tile_ap, col)
                    for (blk0, nblk) in phase_gathers[(sup, c)]:
                        mt = msgp.tile([128, GCAP_BLOCKS, 128], BF16,
                                       tag="m")
                        nc.gpsimd.dma_gather(
                            mt[:, :nblk, :],
                            chunk_view(L, c),
                            gidx_t[:, blk0 * 8:(blk0 + nblk) * 8],
                            num_idxs=nblk * 128,
                            num_idxs_reg=nblk * 128,
                            elem_size=128, single_packet=True,
                            queue_num=gq[0] % 4)
                        gq[0] += 1
                        for j in range(nblk):
                            mt_of[blk0 + j] = (mt, j)
                    # hoist one-hot generation out of the matmul chain
                    for b in sorted(mt_of):
                        get_oh(b)
                    for i in range(nt):
                        t = t0 + i
                        nb = int(nblk_tc[t, c])
                        b0 = int(blk0_tc[t, c])
                        for j in range(nb):
                            mt, col = mt_of[b0 + j]
                            nc.tensor.matmul(
                                acc[:, i, :],
                                get_oh(b0 + j),
                                mt[:, col, 0:D],
                                start=(j == 0),
                                stop=(j == nb - 1))
                    # drain phase accumulator into SBUF
                    if c == first_c:
                        nc.vector.tensor_copy(hv, acc[:, :nt, :])
                    else:
                        nc.vector.tensor_tensor(hv, hv, acc[:, :nt, :],
                                                mybir.AluOpType.add)
                    if c != last_c:
                        continue
                    # epilogue: h = relu(dinv*(h + s_local) + b)
                    nc.vector.tensor_tensor(hv, hv,
                                            s_t[L][:, t0:t0 + nt, 0:D],
                                            mybir.AluOpType.add)
                    dvb = dinv_t[:, t0:t0 + nt].unsqueeze(2).broadcast_to(
                        [128, nt, D])
                    nc.vector.tensor_tensor(hv, hv, dvb,
                                            mybir.AluOpType.mult)
                    bias = (b1_t if L == 0 else b2_t)[:].unsqueeze(
                        1).broadcast_to([128, nt, D])
                    nc.vector.tensor_tensor(hv, hv, bias,
                                            mybir.AluOpType.add)
                    nc.scalar.activation(hv, hv,
                                         mybir.ActivationFunctionType.Relu)
                    if L == 0:
                        # launch layer-2 table half as soon as its tiles done
                        for h in (0, 1):
                            if t0 < half_t0[h] + GHS[h] <= t0 + nt:
                                build_table_l1(h)
                    else:
                        wb = wout_t[:].unsqueeze(1).broadcast_to(
                            [128, nt, D])
                        nc.vector.tensor_tensor(hv, hv, wb,
                                                mybir.AluOpType.mult)
                        nc.vector.tensor_reduce(o_t[:, t0:t0 + nt], hv,
                                                axis=mybir.AxisListType.X,
                                                op=mybir.AluOpType.add)

            for h in (0, 1):
                build_table_l0(h)
            agg_layer(0)
            oh_tiles.clear()
            agg_layer(1)

            nc.vector.tensor_scalar_add(o_t[:], o_t[:], bout_t[:])
            nc.sync.dma_start(out=oute[:], in_=o_t[:])

    nc.compile()
    return nc


_CACHE = {}


def kernel(x, edge_index, batch, W1, b1, W2, b2, Wout, bout, _trace=False):
    x = np.asarray(x, np.float32)
    edge_index = np.asarray(edge_index)
    W1 = np.asarray(W1, np.float32)
    W2 = np.asarray(W2, np.float32)
    b1 = np.asarray(b1, np.float32)
    b2 = np.asarray(b2, np.float32)
    Wout = np.asarray(Wout, np.float32)
    bout = np.asarray(bout, np.float32).reshape(-1)
    N = x.shape[0]

    key = (N, edge_index.shape[1])
    if key not in _CACHE:
        meta, gidx_all, drow_all, deg_tiles = preprocess(edge_index, N)
        nc = build(meta)
        _CACHE[key] = (meta, gidx_all, drow_all, deg_tiles, nc)
    meta, gidx_all, drow_all, deg_tiles, nc = _CACHE[key]
    REAL, NL = meta["REAL"], meta["NL"]

    ident = np.eye(128, dtype=np.float32)
    b1bc = np.tile(b1[None, :], (128, 1)).astype(np.float32)
    b2bc = np.tile(b2[None, :], (128, 1)).astype(np.float32)
    woutbc = np.tile(Wout.reshape(1, -1), (128, 1)).astype(np.float32)
    boutbc = np.full((128, 1), float(bout[0]), np.float32)
    iota = np.tile(np.repeat(np.arange(128, dtype=np.float32), NB_OH)[None, :],
                   (128, 1)).astype(ml_dtypes.bfloat16)

    in_maps = []
    for c in range(NCORES):
        xs = np.zeros((NL, D), np.float32)
        lo, hi = c * REAL, min((c + 1) * REAL, N)
        xs[:hi - lo] = x[lo:hi]
        in_maps.append({
            "xT": np.ascontiguousarray(xs.T),
            "W1": W1, "W2": W2, "b1bc": b1bc, "b2bc": b2bc,
            "woutbc": woutbc, "boutbc": boutbc,
            "deg": deg_tiles[c], "gidx": gidx_all[c], "drow": drow_all[c],
            "iota": iota, "ident": ident,
        })

    res = bass_utils.run_bass_kernel_spmd(
        nc, in_maps, core_ids=list(range(NCORES)), trace=_trace)

    out = np.zeros(N, np.float32)
    for c in range(NCORES):
        o = res.results[c]["out"]
        arr = o.T.ravel()
        lo, hi = c * REAL, min((c + 1) * REAL, N)
        out[lo:hi] = arr[:hi - lo]
    if _trace:
        return out, res.exec_time_ns
    return out
